# revision 18
# baseline (speedup 1.0000x reference)
"""Trainium2 Bass kernel for the 3-layer GAT/GATv2/TransformerConv model.

v4: degree-sorted per-dst transposed edge layout.

Nodes are relabeled so that tile-row t (shared across the 8 cores) holds
nodes of similar in-degree; each tile is [128 dst x J_t edge slots] with
J_t = max degree in the row (padding ~1.2%).  Per-edge source features are
brought in by ONE dma_gather per tile from a 4-node-packed table (row =
256B = 4 nodes x 64B), indexed by src//4 (< 25088, fits int16, no
chunking).  The src%4 sub-row select is done with fused
scalar_tensor_tensor ops ((class==k) * slice).  dst-side terms broadcast
along the free dim for free; segment softmax + segment sum become plain
free-dim reductions.  No one-hot matmuls, no PE transposes in E2/E3.

 - E1 (GATConv 3->4x16): host streams per-edge [x[src]|1|alpha_pre] rows;
   device does leaky/exp/outer-product and reduces along j.
 - E2 (GATv2 64->2x16): gather hl (32 bf16 per node) by src; hr[dst] is
   partition-local.
 - E3 (TransformerConv 32->7): gather kv (14 bf16 used of 32) by src;
   q[dst] partition-local.
 - AllGather of the packed bf16 tables between layers (6.4MB each).
"""
import numpy as np
import ml_dtypes

import concourse.bass as bass
import concourse.bacc as bacc
import concourse.mybir as mybir
import concourse.tile as tile
from concourse.bass_utils import run_bass_kernel_spmd
from concourse.masks import make_identity

dt = mybir.dt

N = 100000
E = 1600000
NCORES = 8
P = 128
NT = 98                 # node tiles per core
NPC = NT * P            # nodes per core (12544)
NPAD = NPC * NCORES     # 100352
TROW = P * NCORES       # 1024: ranks per tile-row
NQ = 4                  # SWDGE queues
BL = 7                  # tiles per load/store batch (98 = 14*7)

F32MAX = np.float32(3.0e38)


# ---------------------------------------------------------------- host prep

def wrap16(idx_flat):
    """dma_gather index layout: idx i -> partition i%16, col i//16,
    replicated across the 8 GPSIMD cores (128 partitions)."""
    n = len(idx_flat)
    a = np.asarray(idx_flat, dtype=np.int16).reshape(n // 16, 16).T.copy()
    return np.tile(a, (8, 1))


def edge_geometry(gsrc, gdst, deg_g):
    """Per-dst transposed slot layout.

    Returns (J [NT], off [NT+1], and per-core slot fill info)."""
    J = deg_g.reshape(NCORES, NT, P).max(axis=(0, 2)).astype(np.int64)
    J = np.maximum(J, 1)
    off = np.concatenate([[0], np.cumsum(J)])
    SJ = int(off[-1])
    o = np.argsort(gdst, kind="stable")
    gs, gd = gsrc[o], gdst[o]
    starts = np.searchsorted(gd, np.arange(NPAD))
    j_e = np.arange(len(gd)) - starts[gd]
    t_e = (gd % NPC) // P
    p_e = gd % P
    c_e = gd // NPC
    col_e = off[t_e] + j_e
    return dict(J=J, off=off, SJ=SJ, gs=gs, gd=gd, c_e=c_e, p_e=p_e,
                col_e=col_e)


def pack_core(geo, c, xg=None, aprg=None):
    """Build per-core device arrays for one edge set.

    Returns dict with src16 [128, SJ*8], cls [128, SJ] bf16, and (if
    xg/aprg given) xa [128, 8*SJ] bf16 in per-tile [8, J_t] blocks."""
    J, off, SJ = geo["J"], geo["off"], geo["SJ"]
    sel = geo["c_e"] == c
    gs = geo["gs"][sel]
    pos = geo["col_e"][sel] * 128 + geo["p_e"][sel]

    idxflat = np.zeros(SJ * 128, dtype=np.int32)
    clsflat = np.full(SJ * 128, 100.0, dtype=np.float32)
    idxflat[pos] = gs // 4
    clsflat[pos] = gs % 4

    src16 = np.empty((128, SJ * 8), dtype=np.int16)
    for t in range(NT):
        a, b = int(off[t]), int(off[t + 1])
        src16[:, a * 8:b * 8] = wrap16(idxflat[a * 128:b * 128].astype(np.int16))
    cls = np.ascontiguousarray(clsflat.reshape(SJ, 128).T).astype(
        ml_dtypes.bfloat16)
    out = dict(src16=src16, cls=cls)

    if xg is not None:
        gd = geo["gd"][sel]
        xa8 = np.zeros((SJ * 128, 8), dtype=np.float32)
        xa8[pos, 0:3] = xg[gs]
        xa8[pos, 3] = 1.0
        xa8[pos, 4:8] = aprg[0][gs] + aprg[1][gd]
        xa = np.empty((128, 8 * SJ), dtype=np.float32)
        for t in range(NT):
            a, b = int(off[t]), int(off[t + 1])
            blk = xa8[a * 128:b * 128].reshape(b - a, 128, 8)
            xa[:, a * 8:b * 8] = blk.transpose(1, 2, 0).reshape(128, -1)
        out["xa"] = xa.astype(ml_dtypes.bfloat16)
    return out


# ---------------------------------------------------------------- bass build

def build(J12, off12, J3, off3):
    f32, bf16, i16 = dt.float32, dt.bfloat16, dt.int16
    nc = bacc.Bacc("TRN2", target_bir_lowering=False, debug=False,
                   num_devices=NCORES, num_swdge_queues=NQ,
                   dynamic_dma_scratch_size=65536)
    SJ12, SJ3 = int(off12[-1]), int(off3[-1])
    Jm12 = int(J12.max())
    Jm3 = int(J3.max())
    STT = mybir.AluOpType
    CP = mybir.ActivationFunctionType.Copy
    EXP = mybir.ActivationFunctionType.Exp

    Wblk = nc.dram_tensor("Wblk", [16, 64], f32, kind="ExternalInput")
    W2l = nc.dram_tensor("W2l", [64, 32], f32, kind="ExternalInput")
    W2r = nc.dram_tensor("W2r", [64, 32], f32, kind="ExternalInput")
    b1r = nc.dram_tensor("b1r", [P, 64], f32, kind="ExternalInput")
    b2lr = nc.dram_tensor("b2lr", [P, 32], f32, kind="ExternalInput")
    b2rr = nc.dram_tensor("b2rr", [P, 32], f32, kind="ExternalInput")
    b2r_ = nc.dram_tensor("b2r_", [P, 32], f32, kind="ExternalInput")
    att2r = nc.dram_tensor("att2r", [P, 32], f32, kind="ExternalInput")
    Wqkvs = nc.dram_tensor("Wqkvs", [32, 28], f32, kind="ExternalInput")
    bqkvsr = nc.dram_tensor("bqkvsr", [P, 28], f32, kind="ExternalInput")
    xaf = nc.dram_tensor("xaf", [P, 8 * SJ12], bf16, kind="ExternalInput")
    src12f = nc.dram_tensor("src12f", [128, SJ12 * 8], i16,
                            kind="ExternalInput")
    cls12f = nc.dram_tensor("cls12f", [P, SJ12], bf16, kind="ExternalInput")
    src3f = nc.dram_tensor("src3f", [128, SJ3 * 8], i16,
                           kind="ExternalInput")
    cls3f = nc.dram_tensor("cls3f", [P, SJ3], bf16, kind="ExternalInput")
    out_loc = nc.dram_tensor("out_loc", [NPC, 7], f32, kind="ExternalOutput")

    # packed tables: 4 nodes per 256B row
    hl_loc = nc.dram_tensor("hl_loc", [NPC, 32], bf16, kind="Internal")
    hl_full = nc.dram_tensor("hl_full", [NPAD, 32], bf16, kind="Internal",
                             addr_space="Shared")
    kv_loc = nc.dram_tensor("kv_loc", [NPC, 32], bf16, kind="Internal")
    kv_full = nc.dram_tensor("kv_full", [NPAD, 32], bf16, kind="Internal",
                             addr_space="Shared")
    hl_view = hl_full.ap().rearrange("(r k) c -> r (k c)", k=4)
    kv_view = kv_full.ap().rearrange("(r k) c -> r (k c)", k=4)

    RG = [[i for i in range(NCORES)]]
    BYP = mybir.AluOpType.bypass
    scale3 = 1.0 / float(np.sqrt(np.float32(7.0)))
    GC = 37                 # gather sub-call size (cols)

    def split_gather(out_tile, view, sib, lo, J, t):
        q = t
        for g0 in range(0, J, GC):
            gn = min(GC, J - g0)
            nc.gpsimd.dma_gather(
                out_ap=out_tile[:, g0:g0 + gn, :], in_ap=view,
                idxs_ap=sib[:, 8 * (lo + g0):8 * (lo + g0 + gn)],
                num_idxs=gn * 128, num_idxs_reg=gn * 128,
                elem_size=128, single_packet=False, queue_num=q % NQ)
            q += 1

    def tree_reduce(msg, acc, J):
        """Halving sum over the middle (j) dim of bf16 msg [P, J, C] into
        f32 acc [P, ceil(J/2), C]; total lands in acc[:, 0:1, :]."""
        h = J // 2
        if h == 0:
            nc.vector.tensor_copy(out=acc[:, 0:1, :], in_=msg[:, 0:1, :])
            return
        nc.vector.tensor_add(out=acc[:, 0:h, :], in0=msg[:, 0:h, :],
                             in1=msg[:, J - h:J, :])
        n = h
        if J % 2:
            # scalar-add instead of copy: mixed-dtype CAST copies are slow
            nc.vector.tensor_scalar_add(acc[:, h:h + 1, :],
                                        msg[:, h:h + 1, :], 0.0)
            n = h + 1
        while n > 1:
            h = n // 2
            nc.vector.tensor_add(out=acc[:, 0:h, :], in0=acc[:, 0:h, :],
                                 in1=acc[:, n - h:n, :])
            n = n - h

    with tile.TileContext(nc) as tc:
        with tc.tile_pool(name="cst", bufs=1) as cst:
            ident = cst.tile([P, P], f32)
            make_identity(nc, ident[:])

            def const(name, t, shape, dtype=f32):
                s = cst.tile(shape, dtype, tag=name)
                nc.sync.dma_start(out=s[:], in_=t.ap())
                return s
            wblk_sb = const("wblk", Wblk, [16, 64])
            w2l_sb = const("w2l", W2l, [64, 32])
            w2r_sb = const("w2r", W2r, [64, 32])
            wqkvs_sb = const("wqkvs", Wqkvs, [32, 28])
            b1_sb = const("b1", b1r, [P, 64])
            b2l_sb = const("b2l", b2lr, [P, 32])
            b2r_sb = const("b2r", b2rr, [P, 32])
            b2_sb = const("b2", b2r_, [P, 32])
            att2_f = const("att2f", att2r, [P, 32])
            att2_sb = cst.tile([P, 32], bf16, tag="att2")
            nc.vector.tensor_copy(out=att2_sb[:], in_=att2_f[:])
            bqkvs_sb = const("bqkvs", bqkvsr, [P, 28])
            # persistent dst-side per-tile state
            hr_all = cst.tile([P, NT, 32], f32, tag="hr_all")
            q_all = cst.tile([P, NT, 7], f32, tag="q_all")
            skip_sb = cst.tile([P, NT, 7], f32, tag="skip")

            # ---------------- E1: GATConv(3, 16, heads=4) ----------------
            with nc.named_scope("E1"), \
                 tc.tile_pool(name="e1", bufs=3) as pe, \
                 tc.tile_pool(name="e1l", bufs=2) as pl, \
                 tc.tile_pool(name="e1x", bufs=2, space="PSUM") as ppx, \
                 tc.tile_pool(name="e1y", bufs=2, space="PSUM") as ppy:
                for tb in range(0, NT, BL):
                    o0, o1 = int(off12[tb]), int(off12[tb + BL])
                    xab = pl.tile([P, 8 * (Jm12 * BL)], bf16, tag="xab")
                    nc.sync.dma_start(out=xab[:, 0:8 * (o1 - o0)],
                                      in_=xaf.ap()[:, 8 * o0:8 * o1])
                    hlst = pl.tile([P, BL, 32], bf16, tag="hlst")
                    for bi in range(BL):
                        t = tb + bi
                        J = int(J12[t])
                        lo = 8 * (int(off12[t]) - o0)
                        xa = xab[:, lo:lo + 8 * J].rearrange(
                            "p (k j) -> p k j", k=8)
                        al = pe.tile([P, 4, Jm12], bf16, tag="al")
                        nc.vector.scalar_tensor_tensor(
                            out=al[:, :, 0:J], in0=xa[:, 4:8, :], scalar=0.2,
                            in1=xa[:, 4:8, :], op0=STT.mult, op1=STT.max)
                        exb = pe.tile([P, 4, Jm12], bf16, tag="exb")
                        nc.scalar.activation(out=exb[:, :, 0:J],
                                             in_=al[:, :, 0:J], func=EXP)
                        xe = pe.tile([P, 16, Jm12], bf16, tag="xe")
                        nc.vector.tensor_mul(
                            out=xe[:, :, 0:J].rearrange(
                                "p (h k) j -> p h k j", h=4),
                            in0=exb[:, :, None, 0:J].to_broadcast(
                                [P, 4, 4, J]),
                            in1=xa[:, None, 0:4, :].to_broadcast(
                                [P, 4, 4, J]))
                        A_sb = pe.tile([P, 16], f32, tag="A")
                        nc.vector.tensor_reduce(
                            out=A_sb[:], in_=xe[:, :, 0:J],
                            axis=mybir.AxisListType.X, op=STT.add)
                        den = pe.tile([P, 4], f32, tag="den")
                        nc.vector.tensor_scalar_add(
                            den[:],
                            A_sb[:].rearrange("p (h k) -> p h k", k=4)[:, :, 3],
                            1e-16)
                        rden = pe.tile([P, 4], f32, tag="rden")
                        nc.vector.reciprocal(out=rden[:], in_=den[:])
                        A_T_ps = ppx.tile([16, P], f32, tag="AT")
                        nc.tensor.transpose(out=A_T_ps[:], in_=A_sb[:],
                                            identity=ident[:])
                        A_T = pe.tile([16, P], f32, tag="ATs")
                        nc.scalar.activation(out=A_T[:], in_=A_T_ps[:],
                                             func=CP)
                        x1_ps = ppx.tile([P, 64], f32, tag="x1p")
                        nc.tensor.matmul(out=x1_ps[:], lhsT=A_T[:],
                                         rhs=wblk_sb[:], start=True, stop=True)
                        x1t = pe.tile([P, 64], f32, tag="x1t")
                        nc.vector.tensor_mul(
                            out=x1t[:].rearrange("p (h c) -> p h c", h=4),
                            in0=x1_ps[:].rearrange("p (h c) -> p h c", h=4),
                            in1=rden[:, :, None].to_broadcast([P, 4, 16]))
                        nc.vector.tensor_add(out=x1t[:], in0=x1t[:],
                                             in1=b1_sb[:])
                        nc.vector.tensor_scalar_max(x1t[:], x1t[:], 0.0)
                        x1T_ps = ppy.tile([64, P], f32, tag="x1T")
                        nc.tensor.transpose(out=x1T_ps[:], in_=x1t[:],
                                            identity=ident[:])
                        x1T = pe.tile([64, P], f32, tag="x1Ts")
                        nc.scalar.activation(out=x1T[:], in_=x1T_ps[:],
                                             func=CP)
                        hlr_ps = ppy.tile([P, 2, 32], f32, tag="hlr")
                        nc.tensor.matmul(out=hlr_ps[:, 0, :], lhsT=x1T[:],
                                         rhs=w2l_sb[:], start=True, stop=True)
                        nc.tensor.matmul(out=hlr_ps[:, 1, :], lhsT=x1T[:],
                                         rhs=w2r_sb[:], start=True, stop=True)
                        nc.vector.tensor_add(out=hlst[:, bi, :],
                                             in0=hlr_ps[:, 0, :],
                                             in1=b2l_sb[:])
                        nc.vector.tensor_add(out=hr_all[:, t, :],
                                             in0=hlr_ps[:, 1, :],
                                             in1=b2r_sb[:])
                    nc.sync.dma_start(
                        out=hl_loc.ap()[tb * P:(tb + BL) * P, :].rearrange(
                            "(b p) c -> p b c", p=P),
                        in_=hlst[:])
            tc.strict_bb_all_engine_barrier()
            nc.gpsimd.collective_compute("AllGather", BYP, RG,
                                         ins=[hl_loc.ap()],
                                         outs=[hl_full.ap()])
            tc.strict_bb_all_engine_barrier()

            # ---------------- E2: GATv2Conv(64, 16, heads=2) ----------------
            with nc.named_scope("E2"), \
                 tc.tile_pool(name="e2", bufs=3) as pe, \
                 tc.tile_pool(name="e2l", bufs=2) as pl, \
                 tc.tile_pool(name="e2x", bufs=2, space="PSUM") as ppx:
                for tb in range(0, NT, BL):
                    o0, o1 = int(off12[tb]), int(off12[tb + BL])
                    sib = pl.tile([P, 8 * Jm12 * BL], i16, tag="sib")
                    nc.sync.dma_start(out=sib[:, 0:8 * (o1 - o0)],
                                      in_=src12f.ap()[:, 8 * o0:8 * o1])
                    clsb = pl.tile([P, Jm12 * BL], bf16, tag="clsb")
                    nc.sync.dma_start(out=clsb[:, 0:o1 - o0],
                                      in_=cls12f.ap()[:, o0:o1])
                    kvst = pl.tile([P, BL, 32], bf16, tag="kvst")
                    nc.vector.memset(kvst[:, :, 14:32], 0.0)
                    for bi in range(BL):
                        t = tb + bi
                        J = int(J12[t])
                        lo = int(off12[t]) - o0
                        cls = clsb[:, lo:lo + J]
                        hlg = pe.tile([P, Jm12, 128], bf16, tag="hlg")
                        split_gather(hlg, hl_view, sib, lo, J, t)
                        # src%4 sub-row select: hlc = sum_k (cls==k)*hlg_k
                        hk = [pe.tile([P, Jm12, 32], bf16, tag=f"hk{k}",
                                      name=f"hk{k}")
                              for k in range(4)]
                        for k in range(4):
                            nc.vector.scalar_tensor_tensor(
                                out=hk[k][:, 0:J, :],
                                in0=cls[:, :, None].to_broadcast([P, J, 32]),
                                scalar=float(k), op0=STT.is_equal,
                                op1=STT.mult,
                                in1=hlg[:, 0:J, 32 * k:32 * k + 32])
                        nc.vector.tensor_add(out=hk[0][:, 0:J, :],
                                             in0=hk[0][:, 0:J, :],
                                             in1=hk[1][:, 0:J, :])
                        nc.vector.tensor_add(out=hk[2][:, 0:J, :],
                                             in0=hk[2][:, 0:J, :],
                                             in1=hk[3][:, 0:J, :])
                        hlc = pe.tile([P, Jm12, 32], bf16, tag="hlc")
                        nc.vector.tensor_add(out=hlc[:, 0:J, :],
                                             in0=hk[0][:, 0:J, :],
                                             in1=hk[2][:, 0:J, :])
                        es = pe.tile([P, Jm12, 32], bf16, tag="es")
                        nc.vector.tensor_add(
                            out=es[:, 0:J, :], in0=hlc[:, 0:J, :],
                            in1=hr_all[:, t:t + 1, :].to_broadcast([P, J, 32]))
                        nc.vector.scalar_tensor_tensor(
                            out=es[:, 0:J, :], in0=es[:, 0:J, :], scalar=0.2,
                            in1=es[:, 0:J, :], op0=STT.mult, op1=STT.max)
                        nc.vector.tensor_mul(
                            out=es[:, 0:J, :], in0=es[:, 0:J, :],
                            in1=att2_sb[:, None, :].to_broadcast([P, J, 32]))
                        # att-dot via in-place halving over c (tensor_reduce
                        # with [P,J,h] output measured pathologically slow)
                        esv = es[:, 0:J, :].rearrange(
                            "p j (h c) -> p j h c", h=2)
                        for w in (8, 4, 2):
                            nc.vector.tensor_add(out=esv[:, :, :, 0:w],
                                                 in0=esv[:, :, :, 0:w],
                                                 in1=esv[:, :, :, w:2 * w])
                        al = pe.tile([P, Jm12, 2], f32, tag="al2")
                        nc.vector.tensor_add(out=al[:, 0:J, :, None],
                                             in0=esv[:, :, :, 0:1],
                                             in1=esv[:, :, :, 1:2])
                        exb = pe.tile([P, Jm12, 2], bf16, tag="exb2")
                        nc.scalar.activation(out=exb[:, 0:J, :],
                                             in_=al[:, 0:J, :], func=EXP)
                        msg = pe.tile([P, Jm12, 34], bf16, tag="msg")
                        nc.vector.scalar_tensor_tensor(
                            out=msg[:, 0:J, 32:34],
                            in0=cls[:, :, None].to_broadcast([P, J, 2]),
                            scalar=50.0, op0=STT.is_lt, op1=STT.mult,
                            in1=exb[:, 0:J, :])
                        nc.vector.tensor_mul(
                            out=msg[:, 0:J, 0:32].rearrange(
                                "p j (h c) -> p j h c", h=2),
                            in0=hlc[:, 0:J, :].rearrange(
                                "p j (h c) -> p j h c", h=2),
                            in1=msg[:, 0:J, 32:34, None].to_broadcast(
                                [P, J, 2, 16]))
                        acc = pe.tile([P, (Jm12 + 1) // 2, 34], f32, tag="acc")
                        tree_reduce(msg[:, 0:J, :], acc, J)
                        den = pe.tile([P, 2], f32, tag="den2")
                        nc.vector.tensor_scalar_add(den[:], acc[:, 0, 32:34],
                                                    1e-16)
                        rden = pe.tile([P, 2], f32, tag="rden2")
                        nc.vector.reciprocal(out=rden[:], in_=den[:])
                        x2t = pe.tile([P, 32], f32, tag="x2t")
                        nc.vector.tensor_mul(
                            out=x2t[:].rearrange("p (h c) -> p h c", h=2),
                            in0=acc[:, 0, 0:32].rearrange(
                                "p (h c) -> p h c", h=2),
                            in1=rden[:, :, None].to_broadcast([P, 2, 16]))
                        nc.vector.tensor_add(out=x2t[:], in0=x2t[:],
                                             in1=b2_sb[:])
                        x2T_ps = ppx.tile([32, P], f32, tag="x2T")
                        nc.tensor.transpose(out=x2T_ps[:], in_=x2t[:],
                                            identity=ident[:])
                        x2T = pe.tile([32, P], f32, tag="x2Ts")
                        nc.scalar.activation(out=x2T[:], in_=x2T_ps[:],
                                             func=CP)
                        qk_ps = ppx.tile([P, 28], f32, tag="qkp")
                        nc.tensor.matmul(out=qk_ps[:], lhsT=x2T[:],
                                         rhs=wqkvs_sb[:], start=True,
                                         stop=True)
                        nc.vector.tensor_add(out=q_all[:, t, :],
                                             in0=qk_ps[:, 0:7],
                                             in1=bqkvs_sb[:, 0:7])
                        nc.vector.tensor_add(out=kvst[:, bi, 0:14],
                                             in0=qk_ps[:, 7:21],
                                             in1=bqkvs_sb[:, 7:21])
                        nc.vector.tensor_add(out=skip_sb[:, t, :],
                                             in0=qk_ps[:, 21:28],
                                             in1=bqkvs_sb[:, 21:28])
                    nc.sync.dma_start(
                        out=kv_loc.ap()[tb * P:(tb + BL) * P, :].rearrange(
                            "(b p) c -> p b c", p=P),
                        in_=kvst[:])
            tc.strict_bb_all_engine_barrier()
            nc.gpsimd.collective_compute("AllGather", BYP, RG,
                                         ins=[kv_loc.ap()],
                                         outs=[kv_full.ap()])
            tc.strict_bb_all_engine_barrier()

            # ---------------- E3: TransformerConv(32, 7) ----------------
            with nc.named_scope("E3"), \
                 tc.tile_pool(name="e3", bufs=3) as pe, \
                 tc.tile_pool(name="e3l", bufs=2) as pl:
                for tb in range(0, NT, BL):
                    o0, o1 = int(off3[tb]), int(off3[tb + BL])
                    sib = pl.tile([P, 8 * Jm3 * BL], i16, tag="sib")
                    nc.sync.dma_start(out=sib[:, 0:8 * (o1 - o0)],
                                      in_=src3f.ap()[:, 8 * o0:8 * o1])
                    clsb = pl.tile([P, Jm3 * BL], bf16, tag="clsb")
                    nc.sync.dma_start(out=clsb[:, 0:o1 - o0],
                                      in_=cls3f.ap()[:, o0:o1])
                    x3st = pl.tile([P, BL, 7], f32, tag="x3st")
                    for bi in range(BL):
                        t = tb + bi
                        J = int(J3[t])
                        lo = int(off3[t]) - o0
                        cls = clsb[:, lo:lo + J]
                        kvg = pe.tile([P, Jm3, 128], bf16, tag="kvg")
                        split_gather(kvg, kv_view, sib, lo, J, t)
                        hk = [pe.tile([P, Jm3, 14], bf16, tag=f"kk{k}",
                                      name=f"kk{k}")
                              for k in range(4)]
                        for k in range(4):
                            nc.vector.scalar_tensor_tensor(
                                out=hk[k][:, 0:J, :],
                                in0=cls[:, :, None].to_broadcast([P, J, 14]),
                                scalar=float(k), op0=STT.is_equal,
                                op1=STT.mult,
                                in1=kvg[:, 0:J, 32 * k:32 * k + 14])
                        nc.vector.tensor_add(out=hk[0][:, 0:J, :],
                                             in0=hk[0][:, 0:J, :],
                                             in1=hk[1][:, 0:J, :])
                        nc.vector.tensor_add(out=hk[2][:, 0:J, :],
                                             in0=hk[2][:, 0:J, :],
                                             in1=hk[3][:, 0:J, :])
                        kvc = pe.tile([P, Jm3, 14], bf16, tag="kvc")
                        nc.vector.tensor_add(out=kvc[:, 0:J, :],
                                             in0=hk[0][:, 0:J, :],
                                             in1=hk[2][:, 0:J, :])
                        qk = pe.tile([P, Jm3, 7], bf16, tag="qk")
                        nc.vector.tensor_mul(
                            out=qk[:, 0:J, :], in0=kvc[:, 0:J, 0:7],
                            in1=q_all[:, t:t + 1, :].to_broadcast([P, J, 7]))
                        # q.k dot via in-place halving over c (7 = 3+4)
                        nc.vector.tensor_add(out=qk[:, 0:J, 0:3],
                                             in0=qk[:, 0:J, 0:3],
                                             in1=qk[:, 0:J, 4:7])
                        nc.vector.tensor_add(out=qk[:, 0:J, 0:2],
                                             in0=qk[:, 0:J, 0:2],
                                             in1=qk[:, 0:J, 2:4])
                        al = pe.tile([P, Jm3, 1], f32, tag="al3")
                        nc.vector.tensor_add(out=al[:, 0:J, :],
                                             in0=qk[:, 0:J, 0:1],
                                             in1=qk[:, 0:J, 1:2])
                        exb = pe.tile([P, Jm3, 1], bf16, tag="exb3")
                        nc.scalar.activation(out=exb[:, 0:J, :],
                                             in_=al[:, 0:J, :], func=EXP,
                                             scale=scale3)
                        msg = pe.tile([P, Jm3, 8], bf16, tag="msg3")
                        nc.vector.scalar_tensor_tensor(
                            out=msg[:, 0:J, 7:8],
                            in0=cls[:, :, None].to_broadcast([P, J, 1]),
                            scalar=50.0, op0=STT.is_lt, op1=STT.mult,
                            in1=exb[:, 0:J, :])
                        nc.vector.tensor_mul(
                            out=msg[:, 0:J, 0:7], in0=kvc[:, 0:J, 7:14],
                            in1=msg[:, 0:J, 7:8].to_broadcast([P, J, 7]))
                        acc = pe.tile([P, (Jm3 + 1) // 2, 8], f32, tag="acc3")
                        tree_reduce(msg[:, 0:J, :], acc, J)
                        den = pe.tile([P, 1], f32, tag="den3")
                        nc.vector.tensor_scalar_add(den[:], acc[:, 0, 7:8],
                                                    1e-16)
                        rden = pe.tile([P, 1], f32, tag="rden3")
                        nc.vector.reciprocal(out=rden[:], in_=den[:])
                        nc.vector.tensor_mul(out=x3st[:, bi, :],
                                             in0=acc[:, 0, 0:7],
                                             in1=rden[:].to_broadcast([P, 7]))
                        nc.vector.tensor_add(out=x3st[:, bi, :],
                                             in0=x3st[:, bi, :],
                                             in1=skip_sb[:, t, :])
                    nc.sync.dma_start(
                        out=out_loc.ap()[tb * P:(tb + BL) * P, :].rearrange(
                            "(b p) c -> p b c", p=P),
                        in_=x3st[:])
    nc.compile()
    return nc


# ---------------------------------------------------------------- kernel

def kernel(x, edge_index, W1, att_src1, att_dst1, b1, W2l, b2l, W2r, b2r,
           att2, b2, Wq, bq, Wk, bk, Wv, bv, Wskip, bskip):
    x = np.asarray(x, dtype=np.float32)
    edge_index = np.asarray(edge_index)
    W1 = np.asarray(W1, dtype=np.float64)
    att_src1 = np.asarray(att_src1, dtype=np.float64)
    att_dst1 = np.asarray(att_dst1, dtype=np.float64)

    src = np.asarray(edge_index[0], dtype=np.int64)
    dst = np.asarray(edge_index[1], dtype=np.int64)

    # degree-sorted node relabeling: rank r -> g = c*NPC + t*128 + p
    deg12 = np.bincount(dst, minlength=N) + 1
    order = np.argsort(-deg12, kind="stable")
    r = np.arange(NPAD)
    g_of_rank = ((r % TROW) // P) * NPC + (r // TROW) * P + (r % P)
    g_of_node = np.empty(N, dtype=np.int64)
    g_of_node[order] = g_of_rank[:N]

    gsrc = g_of_node[src]
    gdst = g_of_node[dst]
    gloop = g_of_node[np.arange(N)]
    gsrc12 = np.concatenate([gsrc, gloop])
    gdst12 = np.concatenate([gdst, gloop])

    deg12_g = np.zeros(NPAD, dtype=np.int64)
    deg12_g[g_of_node] = deg12
    deg3_g = np.zeros(NPAD, dtype=np.int64)
    deg3_g[g_of_node] = deg12 - 1

    geo12 = edge_geometry(gsrc12, gdst12, deg12_g)
    geo3 = edge_geometry(gsrc, gdst, deg3_g)

    # host-side E1 projections (as in the reference, fp64 for stability)
    W1r = W1.reshape(3, 4, 16)
    Asrc3 = (W1r * att_src1[None]).sum(-1)    # [3, 4]
    Adst3 = (W1r * att_dst1[None]).sum(-1)
    xg = np.zeros((NPAD, 3), dtype=np.float32)
    xg[g_of_node] = x
    asrc_g = (xg.astype(np.float64) @ Asrc3).astype(np.float32)
    adst_g = (xg.astype(np.float64) @ Adst3).astype(np.float32)

    # block-diagonal W1 for x1 = A @ Wblk: rows (h,k<3) -> cols (h, c)
    Wblk = np.zeros((16, 64), dtype=np.float32)
    for h in range(4):
        Wblk[h * 4:h * 4 + 3, h * 16:(h + 1) * 16] = W1[:, h * 16:(h + 1) * 16]

    rep = lambda v, w: np.broadcast_to(np.asarray(v, np.float32).reshape(1, w),
                                       (P, w)).copy()
    shared = {
        "Wblk": Wblk,
        "W2l": np.asarray(W2l, np.float32),
        "W2r": np.asarray(W2r, np.float32),
        "b1r": rep(b1, 64), "b2lr": rep(b2l, 32), "b2rr": rep(b2r, 32),
        "b2r_": rep(b2, 32),
        "att2r": rep(np.asarray(att2, np.float32).reshape(32), 32),
        "Wqkvs": np.concatenate([Wq, Wk, Wv, Wskip], axis=1).astype(np.float32),
        "bqkvsr": rep(np.concatenate([np.asarray(bq), np.asarray(bk),
                                      np.asarray(bv), np.asarray(bskip)]), 28),
    }
    in_maps = []
    for c in range(NCORES):
        m = dict(shared)
        d12 = pack_core(geo12, c, xg=xg, aprg=(asrc_g, adst_g))
        d3 = pack_core(geo3, c)
        m["xaf"] = d12["xa"]
        m["src12f"] = d12["src16"]
        m["cls12f"] = d12["cls"]
        m["src3f"] = d3["src16"]
        m["cls3f"] = d3["cls"]
        in_maps.append(m)

    nc = build(geo12["J"], geo12["off"], geo3["J"], geo3["off"])
    trace = bool(globals().get("_TRACE", False))
    res = run_bass_kernel_spmd(nc, in_maps, core_ids=list(range(NCORES)),
                               trace=trace)
    if trace:
        globals()["_LAST_RES"] = res
    outg = np.concatenate([res.results[c]["out_loc"] for c in range(NCORES)],
                          axis=0)
    return np.ascontiguousarray(outg[g_of_node]).astype(np.float32)


# revision 24
# speedup vs baseline: 1.7703x; 1.7703x over previous
"""Trainium2 Bass kernel for the 3-layer GAT/GATv2/TransformerConv model.

v4: degree-sorted per-dst transposed edge layout.

Nodes are relabeled so that tile-row t (shared across the 8 cores) holds
nodes of similar in-degree; each tile is [128 dst x J_t edge slots] with
J_t = max degree in the row (padding ~1.2%).  Per-edge source features are
brought in by ONE dma_gather per tile from a 4-node-packed table (row =
256B = 4 nodes x 64B), indexed by src//4 (< 25088, fits int16, no
chunking).  The src%4 sub-row select is done with fused
scalar_tensor_tensor ops ((class==k) * slice).  dst-side terms broadcast
along the free dim for free; segment softmax + segment sum become plain
free-dim reductions.  No one-hot matmuls, no PE transposes in E2/E3.

 - E1 (GATConv 3->4x16): host streams per-edge [x[src]|1|alpha_pre] rows;
   device does leaky/exp/outer-product and reduces along j.
 - E2 (GATv2 64->2x16): gather hl (32 bf16 per node) by src; hr[dst] is
   partition-local.
 - E3 (TransformerConv 32->7): gather kv (14 bf16 used of 32) by src;
   q[dst] partition-local.
 - AllGather of the packed bf16 tables between layers (6.4MB each).
"""
import numpy as np
import ml_dtypes

import concourse.bass as bass
import concourse.bacc as bacc
import concourse.mybir as mybir
import concourse.tile as tile
from concourse.bass_utils import run_bass_kernel_spmd
from concourse.masks import make_identity

dt = mybir.dt

N = 100000
E = 1600000
NCORES = 8
P = 128
NT = 98                 # node tiles per core
NPC = NT * P            # nodes per core (12544)
NPAD = NPC * NCORES     # 100352
TROW = P * NCORES       # 1024: ranks per tile-row
NQ = 4                  # SWDGE queues
BL = 7                  # tiles per load/store batch (98 = 14*7)

F32MAX = np.float32(3.0e38)


# ---------------------------------------------------------------- host prep

def wrap16(idx_flat):
    """dma_gather index layout: idx i -> partition i%16, col i//16,
    replicated across the 8 GPSIMD cores (128 partitions)."""
    n = len(idx_flat)
    a = np.asarray(idx_flat, dtype=np.int16).reshape(n // 16, 16).T.copy()
    return np.tile(a, (8, 1))


def edge_geometry(gsrc, gdst, deg_g):
    """Per-dst transposed slot layout.

    Returns (J [NT], off [NT+1], and per-core slot fill info)."""
    J = deg_g.reshape(NCORES, NT, P).max(axis=(0, 2)).astype(np.int64)
    J = np.maximum(J, 1)
    off = np.concatenate([[0], np.cumsum(J)])
    SJ = int(off[-1])
    o = np.argsort(gdst, kind="stable")
    gs, gd = gsrc[o], gdst[o]
    starts = np.searchsorted(gd, np.arange(NPAD))
    j_e = np.arange(len(gd)) - starts[gd]
    t_e = (gd % NPC) // P
    p_e = gd % P
    c_e = gd // NPC
    col_e = off[t_e] + j_e
    return dict(J=J, off=off, SJ=SJ, gs=gs, gd=gd, c_e=c_e, p_e=p_e,
                col_e=col_e)


def pack_core(geo, c, xg=None, aprg=None):
    """Build per-core device arrays for one edge set.

    Returns dict with src16 [128, SJ*8], cls [128, SJ] bf16, and (if
    xg/aprg given) xa [128, 8*SJ] bf16 in per-tile [8, J_t] blocks."""
    J, off, SJ = geo["J"], geo["off"], geo["SJ"]
    sel = geo["c_e"] == c
    gs = geo["gs"][sel]
    pos = geo["col_e"][sel] * 128 + geo["p_e"][sel]

    idxflat = np.zeros(SJ * 128, dtype=np.int32)
    clsflat = np.full(SJ * 128, 100.0, dtype=np.float32)
    idxflat[pos] = gs // 4
    clsflat[pos] = gs % 4

    src16 = np.empty((128, SJ * 8), dtype=np.int16)
    for t in range(NT):
        a, b = int(off[t]), int(off[t + 1])
        src16[:, a * 8:b * 8] = wrap16(idxflat[a * 128:b * 128].astype(np.int16))
    cls = np.ascontiguousarray(clsflat.reshape(SJ, 128).T).astype(
        ml_dtypes.bfloat16)
    out = dict(src16=src16, cls=cls)

    if xg is not None:
        gd = geo["gd"][sel]
        xa8 = np.zeros((SJ * 128, 8), dtype=np.float32)
        xa8[pos, 0:3] = xg[gs]
        xa8[pos, 3] = 1.0
        xa8[pos, 4:8] = aprg[0][gs] + aprg[1][gd]
        xa = np.empty((128, 8 * SJ), dtype=np.float32)
        for t in range(NT):
            a, b = int(off[t]), int(off[t + 1])
            blk = xa8[a * 128:b * 128].reshape(b - a, 128, 8)
            xa[:, a * 8:b * 8] = blk.transpose(1, 2, 0).reshape(128, -1)
        out["xa"] = xa.astype(ml_dtypes.bfloat16)
    return out


# ---------------------------------------------------------------- bass build

def build(J12, off12, J3, off3):
    f32, bf16, i16 = dt.float32, dt.bfloat16, dt.int16
    nc = bacc.Bacc("TRN2", target_bir_lowering=False, debug=False,
                   num_devices=NCORES, num_swdge_queues=NQ)
    SJ12, SJ3 = int(off12[-1]), int(off3[-1])
    Jm12 = int(J12.max())
    Jm3 = int(J3.max())
    STT = mybir.AluOpType
    CP = mybir.ActivationFunctionType.Copy
    EXP = mybir.ActivationFunctionType.Exp

    Wblk = nc.dram_tensor("Wblk", [16, 64], f32, kind="ExternalInput")
    W2l = nc.dram_tensor("W2l", [64, 32], f32, kind="ExternalInput")
    W2r = nc.dram_tensor("W2r", [64, 32], f32, kind="ExternalInput")
    b1r = nc.dram_tensor("b1r", [P, 64], f32, kind="ExternalInput")
    b2lr = nc.dram_tensor("b2lr", [P, 32], f32, kind="ExternalInput")
    b2rr = nc.dram_tensor("b2rr", [P, 32], f32, kind="ExternalInput")
    b2r_ = nc.dram_tensor("b2r_", [P, 32], f32, kind="ExternalInput")
    att2r = nc.dram_tensor("att2r", [P, 32], f32, kind="ExternalInput")
    Wqkvs = nc.dram_tensor("Wqkvs", [32, 28], f32, kind="ExternalInput")
    bqkvsr = nc.dram_tensor("bqkvsr", [P, 28], f32, kind="ExternalInput")
    xaf = nc.dram_tensor("xaf", [P, 8 * SJ12], bf16, kind="ExternalInput")
    src12f = nc.dram_tensor("src12f", [128, SJ12 * 8], i16,
                            kind="ExternalInput")
    cls12f = nc.dram_tensor("cls12f", [P, SJ12], bf16, kind="ExternalInput")
    src3f = nc.dram_tensor("src3f", [128, SJ3 * 8], i16,
                           kind="ExternalInput")
    cls3f = nc.dram_tensor("cls3f", [P, SJ3], bf16, kind="ExternalInput")
    out_loc = nc.dram_tensor("out_loc", [NPC, 7], f32, kind="ExternalOutput")

    # packed tables: 4 nodes per 256B row
    hl_loc = nc.dram_tensor("hl_loc", [NPC, 32], bf16, kind="Internal")
    hl_full = nc.dram_tensor("hl_full", [NPAD, 32], bf16, kind="Internal",
                             addr_space="Shared")
    kv_loc = nc.dram_tensor("kv_loc", [NPC, 32], bf16, kind="Internal")
    kv_full = nc.dram_tensor("kv_full", [NPAD, 32], bf16, kind="Internal",
                             addr_space="Shared")
    hl_view = hl_full.ap().rearrange("(r k) c -> r (k c)", k=4)
    kv_view = kv_full.ap().rearrange("(r k) c -> r (k c)", k=4)

    RG = [[i for i in range(NCORES)]]
    BYP = mybir.AluOpType.bypass
    scale3 = 1.0 / float(np.sqrt(np.float32(7.0)))
    GC = 7                  # gather sub-call size (cols): 896 descs < fifo cap

    def split_gather(out_tile, view, sib, lo, J, t):
        q = t
        for g0 in range(0, J, GC):
            gn = min(GC, J - g0)
            nc.gpsimd.dma_gather(
                out_ap=out_tile[:, g0:g0 + gn, :], in_ap=view,
                idxs_ap=sib[:, 8 * (lo + g0):8 * (lo + g0 + gn)],
                num_idxs=gn * 128, num_idxs_reg=gn * 128,
                elem_size=128, single_packet=False, queue_num=q % NQ)
            q += 1

    def tree_reduce(msg, acc, J):
        """Halving sum over the middle (j) dim of bf16 msg [P, J, C] into
        f32 acc [P, ceil(J/2), C]; total lands in acc[:, 0:1, :]."""
        h = J // 2
        if h == 0:
            nc.vector.tensor_copy(out=acc[:, 0:1, :], in_=msg[:, 0:1, :])
            return
        nc.vector.tensor_add(out=acc[:, 0:h, :], in0=msg[:, 0:h, :],
                             in1=msg[:, J - h:J, :])
        n = h
        if J % 2:
            # fold the odd leftover with a regular add (tensor_scalar and
            # CAST-copy paths are pathologically slow on this DVE)
            nc.vector.tensor_add(out=acc[:, 0:1, :], in0=acc[:, 0:1, :],
                                 in1=msg[:, h:h + 1, :])
        while n > 1:
            h = n // 2
            nc.vector.tensor_add(out=acc[:, 0:h, :], in0=acc[:, 0:h, :],
                                 in1=acc[:, n - h:n, :])
            n = n - h

    with tile.TileContext(nc) as tc:
        with tc.tile_pool(name="cst", bufs=1) as cst:
            ident = cst.tile([P, P], f32)
            make_identity(nc, ident[:])

            def const(name, t, shape, dtype=f32):
                s = cst.tile(shape, dtype, tag=name)
                nc.sync.dma_start(out=s[:], in_=t.ap())
                return s
            wblk_sb = const("wblk", Wblk, [16, 64])
            w2l_sb = const("w2l", W2l, [64, 32])
            w2r_sb = const("w2r", W2r, [64, 32])
            wqkvs_sb = const("wqkvs", Wqkvs, [32, 28])
            b1_sb = const("b1", b1r, [P, 64])
            b2l_sb = const("b2l", b2lr, [P, 32])
            b2r_sb = const("b2r", b2rr, [P, 32])
            b2_sb = const("b2", b2r_, [P, 32])
            att2_f = const("att2f", att2r, [P, 32])
            att2_sb = cst.tile([P, 32], bf16, tag="att2")
            nc.vector.tensor_copy(out=att2_sb[:], in_=att2_f[:])
            bqkvs_sb = const("bqkvs", bqkvsr, [P, 28])
            # persistent dst-side per-tile state
            hr_all = cst.tile([P, NT, 32], f32, tag="hr_all")
            q_all = cst.tile([P, NT, 7], f32, tag="q_all")
            skip_sb = cst.tile([P, NT, 7], f32, tag="skip")
            eps_sb = cst.tile([P, 2], f32, tag="eps")
            nc.vector.memset(eps_sb[:], 1e-16)

            # ---------------- E1: GATConv(3, 16, heads=4) ----------------
            with nc.named_scope("E1"), \
                 tc.tile_pool(name="e1", bufs=3) as pe, \
                 tc.tile_pool(name="e1l", bufs=2) as pl, \
                 tc.tile_pool(name="e1x", bufs=2, space="PSUM") as ppx, \
                 tc.tile_pool(name="e1y", bufs=2, space="PSUM") as ppy:
                for tb in range(0, NT, BL):
                    o0, o1 = int(off12[tb]), int(off12[tb + BL])
                    xab = pl.tile([P, 8 * (Jm12 * BL)], bf16, tag="xab")
                    nc.sync.dma_start(out=xab[:, 0:8 * (o1 - o0)],
                                      in_=xaf.ap()[:, 8 * o0:8 * o1])
                    hlst = pl.tile([P, BL, 32], bf16, tag="hlst")
                    for bi in range(BL):
                        t = tb + bi
                        J = int(J12[t])
                        lo = 8 * (int(off12[t]) - o0)
                        xa = xab[:, lo:lo + 8 * J].rearrange(
                            "p (k j) -> p k j", k=8)
                        al = pe.tile([P, 4, Jm12], bf16, tag="al")
                        nc.vector.scalar_tensor_tensor(
                            out=al[:, :, 0:J], in0=xa[:, 4:8, :], scalar=0.2,
                            in1=xa[:, 4:8, :], op0=STT.mult, op1=STT.max)
                        exb = pe.tile([P, 4, Jm12], bf16, tag="exb")
                        nc.scalar.activation(out=exb[:, :, 0:J],
                                             in_=al[:, :, 0:J], func=EXP)
                        xe = pe.tile([P, 16, Jm12], bf16, tag="xe")
                        nc.vector.tensor_mul(
                            out=xe[:, :, 0:J].rearrange(
                                "p (h k) j -> p h k j", h=4),
                            in0=exb[:, :, None, 0:J].to_broadcast(
                                [P, 4, 4, J]),
                            in1=xa[:, None, 0:4, :].to_broadcast(
                                [P, 4, 4, J]))
                        A_sb = pe.tile([P, 16], f32, tag="A")
                        nc.vector.tensor_reduce(
                            out=A_sb[:], in_=xe[:, :, 0:J],
                            axis=mybir.AxisListType.X, op=STT.add)
                        den = pe.tile([P, 4], f32, tag="den")
                        nc.vector.tensor_scalar_add(
                            den[:],
                            A_sb[:].rearrange("p (h k) -> p h k", k=4)[:, :, 3],
                            1e-16)
                        rden = pe.tile([P, 4], f32, tag="rden")
                        nc.vector.reciprocal(out=rden[:], in_=den[:])
                        A_T_ps = ppx.tile([16, P], f32, tag="AT")
                        nc.tensor.transpose(out=A_T_ps[:], in_=A_sb[:],
                                            identity=ident[:])
                        A_T = pe.tile([16, P], f32, tag="ATs")
                        nc.scalar.activation(out=A_T[:], in_=A_T_ps[:],
                                             func=CP)
                        x1_ps = ppx.tile([P, 64], f32, tag="x1p")
                        nc.tensor.matmul(out=x1_ps[:], lhsT=A_T[:],
                                         rhs=wblk_sb[:], start=True, stop=True)
                        x1t = pe.tile([P, 64], f32, tag="x1t")
                        nc.vector.tensor_mul(
                            out=x1t[:].rearrange("p (h c) -> p h c", h=4),
                            in0=x1_ps[:].rearrange("p (h c) -> p h c", h=4),
                            in1=rden[:, :, None].to_broadcast([P, 4, 16]))
                        nc.vector.tensor_add(out=x1t[:], in0=x1t[:],
                                             in1=b1_sb[:])
                        nc.vector.tensor_scalar_max(x1t[:], x1t[:], 0.0)
                        x1T_ps = ppy.tile([64, P], f32, tag="x1T")
                        nc.tensor.transpose(out=x1T_ps[:], in_=x1t[:],
                                            identity=ident[:])
                        x1T = pe.tile([64, P], f32, tag="x1Ts")
                        nc.scalar.activation(out=x1T[:], in_=x1T_ps[:],
                                             func=CP)
                        hlr_ps = ppy.tile([P, 2, 32], f32, tag="hlr")
                        nc.tensor.matmul(out=hlr_ps[:, 0, :], lhsT=x1T[:],
                                         rhs=w2l_sb[:], start=True, stop=True)
                        nc.tensor.matmul(out=hlr_ps[:, 1, :], lhsT=x1T[:],
                                         rhs=w2r_sb[:], start=True, stop=True)
                        nc.vector.tensor_add(out=hlst[:, bi, :],
                                             in0=hlr_ps[:, 0, :],
                                             in1=b2l_sb[:])
                        nc.vector.tensor_add(out=hr_all[:, t, :],
                                             in0=hlr_ps[:, 1, :],
                                             in1=b2r_sb[:])
                    nc.sync.dma_start(
                        out=hl_loc.ap()[tb * P:(tb + BL) * P, :].rearrange(
                            "(b p) c -> p b c", p=P),
                        in_=hlst[:])
            tc.strict_bb_all_engine_barrier()
            nc.gpsimd.collective_compute("AllGather", BYP, RG,
                                         ins=[hl_loc.ap()],
                                         outs=[hl_full.ap()])
            tc.strict_bb_all_engine_barrier()

            # ---------------- E2: GATv2Conv(64, 16, heads=2) ----------------
            with nc.named_scope("E2"), \
                 tc.tile_pool(name="e2", bufs=3) as pe, \
                 tc.tile_pool(name="e2l", bufs=2) as pl, \
                 tc.tile_pool(name="e2x", bufs=2, space="PSUM") as ppx:
                for tb in range(0, NT, BL):
                    o0, o1 = int(off12[tb]), int(off12[tb + BL])
                    sib = pl.tile([P, 8 * Jm12 * BL], i16, tag="sib")
                    nc.sync.dma_start(out=sib[:, 0:8 * (o1 - o0)],
                                      in_=src12f.ap()[:, 8 * o0:8 * o1])
                    clsb = pl.tile([P, Jm12 * BL], bf16, tag="clsb")
                    nc.sync.dma_start(out=clsb[:, 0:o1 - o0],
                                      in_=cls12f.ap()[:, o0:o1])
                    kvst = pl.tile([P, BL, 32], bf16, tag="kvst")
                    nc.vector.memset(kvst[:, :, 14:32], 0.0)
                    for bi in range(BL):
                        t = tb + bi
                        J = int(J12[t])
                        lo = int(off12[t]) - o0
                        cls = clsb[:, lo:lo + J]
                        hlg = pe.tile([P, Jm12, 128], bf16, tag="hlg")
                        split_gather(hlg, hl_view, sib, lo, J, t)
                        # src%4 sub-row select: hlc = sum_k (cls==k)*hlg_k
                        hk = [pe.tile([P, Jm12, 32], bf16, tag=f"hk{k}",
                                      name=f"hk{k}")
                              for k in range(4)]
                        for k in range(4):
                            nc.vector.scalar_tensor_tensor(
                                out=hk[k][:, 0:J, :],
                                in0=cls[:, :, None].to_broadcast([P, J, 32]),
                                scalar=float(k), op0=STT.is_equal,
                                op1=STT.mult,
                                in1=hlg[:, 0:J, 32 * k:32 * k + 32])
                        nc.vector.tensor_add(out=hk[0][:, 0:J, :],
                                             in0=hk[0][:, 0:J, :],
                                             in1=hk[1][:, 0:J, :])
                        nc.vector.tensor_add(out=hk[2][:, 0:J, :],
                                             in0=hk[2][:, 0:J, :],
                                             in1=hk[3][:, 0:J, :])
                        hlc = pe.tile([P, Jm12, 32], bf16, tag="hlc")
                        nc.vector.tensor_add(out=hlc[:, 0:J, :],
                                             in0=hk[0][:, 0:J, :],
                                             in1=hk[2][:, 0:J, :])
                        es = pe.tile([P, Jm12, 32], bf16, tag="es")
                        nc.vector.tensor_add(
                            out=es[:, 0:J, :], in0=hlc[:, 0:J, :],
                            in1=hr_all[:, t:t + 1, :].to_broadcast([P, J, 32]))
                        nc.vector.scalar_tensor_tensor(
                            out=es[:, 0:J, :], in0=es[:, 0:J, :], scalar=0.2,
                            in1=es[:, 0:J, :], op0=STT.mult, op1=STT.max)
                        nc.vector.tensor_mul(
                            out=es[:, 0:J, :], in0=es[:, 0:J, :],
                            in1=att2_sb[:, None, :].to_broadcast([P, J, 32]))
                        # att-dot via in-place halving over c (tensor_reduce
                        # with [P,J,h] output measured pathologically slow)
                        esv = es[:, 0:J, :].rearrange(
                            "p j (h c) -> p j h c", h=2)
                        for w in (8, 4, 2):
                            nc.vector.tensor_add(out=esv[:, :, :, 0:w],
                                                 in0=esv[:, :, :, 0:w],
                                                 in1=esv[:, :, :, w:2 * w])
                        al = pe.tile([P, Jm12, 2], f32, tag="al2")
                        nc.vector.tensor_add(out=al[:, 0:J, :, None],
                                             in0=esv[:, :, :, 0:1],
                                             in1=esv[:, :, :, 1:2])
                        exb = pe.tile([P, Jm12, 2], bf16, tag="exb2")
                        nc.scalar.activation(out=exb[:, 0:J, :],
                                             in_=al[:, 0:J, :], func=EXP)
                        msg = pe.tile([P, Jm12, 34], bf16, tag="msg")
                        nc.vector.scalar_tensor_tensor(
                            out=msg[:, 0:J, 32:34],
                            in0=cls[:, :, None].to_broadcast([P, J, 2]),
                            scalar=50.0, op0=STT.is_lt, op1=STT.mult,
                            in1=exb[:, 0:J, :])
                        nc.vector.tensor_mul(
                            out=msg[:, 0:J, 0:32].rearrange(
                                "p j (h c) -> p j h c", h=2),
                            in0=hlc[:, 0:J, :].rearrange(
                                "p j (h c) -> p j h c", h=2),
                            in1=msg[:, 0:J, 32:34, None].to_broadcast(
                                [P, J, 2, 16]))
                        acc = pe.tile([P, (Jm12 + 1) // 2, 34], f32, tag="acc")
                        tree_reduce(msg[:, 0:J, :], acc, J)
                        den = pe.tile([P, 2], f32, tag="den2")
                        nc.vector.tensor_add(out=den[:], in0=acc[:, 0, 32:34],
                                             in1=eps_sb[:])
                        rden = pe.tile([P, 2], f32, tag="rden2")
                        nc.vector.reciprocal(out=rden[:], in_=den[:])
                        x2t = pe.tile([P, 32], f32, tag="x2t")
                        nc.vector.tensor_mul(
                            out=x2t[:].rearrange("p (h c) -> p h c", h=2),
                            in0=acc[:, 0, 0:32].rearrange(
                                "p (h c) -> p h c", h=2),
                            in1=rden[:, :, None].to_broadcast([P, 2, 16]))
                        nc.vector.tensor_add(out=x2t[:], in0=x2t[:],
                                             in1=b2_sb[:])
                        x2T_ps = ppx.tile([32, P], f32, tag="x2T")
                        nc.tensor.transpose(out=x2T_ps[:], in_=x2t[:],
                                            identity=ident[:])
                        x2T = pe.tile([32, P], f32, tag="x2Ts")
                        nc.scalar.activation(out=x2T[:], in_=x2T_ps[:],
                                             func=CP)
                        qk_ps = ppx.tile([P, 28], f32, tag="qkp")
                        nc.tensor.matmul(out=qk_ps[:], lhsT=x2T[:],
                                         rhs=wqkvs_sb[:], start=True,
                                         stop=True)
                        nc.vector.tensor_add(out=q_all[:, t, :],
                                             in0=qk_ps[:, 0:7],
                                             in1=bqkvs_sb[:, 0:7])
                        nc.vector.tensor_add(out=kvst[:, bi, 0:14],
                                             in0=qk_ps[:, 7:21],
                                             in1=bqkvs_sb[:, 7:21])
                        nc.vector.tensor_add(out=skip_sb[:, t, :],
                                             in0=qk_ps[:, 21:28],
                                             in1=bqkvs_sb[:, 21:28])
                    nc.sync.dma_start(
                        out=kv_loc.ap()[tb * P:(tb + BL) * P, :].rearrange(
                            "(b p) c -> p b c", p=P),
                        in_=kvst[:])
            tc.strict_bb_all_engine_barrier()
            nc.gpsimd.collective_compute("AllGather", BYP, RG,
                                         ins=[kv_loc.ap()],
                                         outs=[kv_full.ap()])
            tc.strict_bb_all_engine_barrier()

            # ---------------- E3: TransformerConv(32, 7) ----------------
            with nc.named_scope("E3"), \
                 tc.tile_pool(name="e3", bufs=3) as pe, \
                 tc.tile_pool(name="e3l", bufs=2) as pl:
                for tb in range(0, NT, BL):
                    o0, o1 = int(off3[tb]), int(off3[tb + BL])
                    sib = pl.tile([P, 8 * Jm3 * BL], i16, tag="sib")
                    nc.sync.dma_start(out=sib[:, 0:8 * (o1 - o0)],
                                      in_=src3f.ap()[:, 8 * o0:8 * o1])
                    clsb = pl.tile([P, Jm3 * BL], bf16, tag="clsb")
                    nc.sync.dma_start(out=clsb[:, 0:o1 - o0],
                                      in_=cls3f.ap()[:, o0:o1])
                    x3st = pl.tile([P, BL, 7], f32, tag="x3st")
                    for bi in range(BL):
                        t = tb + bi
                        J = int(J3[t])
                        lo = int(off3[t]) - o0
                        cls = clsb[:, lo:lo + J]
                        kvg = pe.tile([P, Jm3, 128], bf16, tag="kvg")
                        split_gather(kvg, kv_view, sib, lo, J, t)
                        hk = [pe.tile([P, Jm3, 14], bf16, tag=f"kk{k}",
                                      name=f"kk{k}")
                              for k in range(4)]
                        for k in range(4):
                            nc.vector.scalar_tensor_tensor(
                                out=hk[k][:, 0:J, :],
                                in0=cls[:, :, None].to_broadcast([P, J, 14]),
                                scalar=float(k), op0=STT.is_equal,
                                op1=STT.mult,
                                in1=kvg[:, 0:J, 32 * k:32 * k + 14])
                        nc.vector.tensor_add(out=hk[0][:, 0:J, :],
                                             in0=hk[0][:, 0:J, :],
                                             in1=hk[1][:, 0:J, :])
                        nc.vector.tensor_add(out=hk[2][:, 0:J, :],
                                             in0=hk[2][:, 0:J, :],
                                             in1=hk[3][:, 0:J, :])
                        kvc = pe.tile([P, Jm3, 14], bf16, tag="kvc")
                        nc.vector.tensor_add(out=kvc[:, 0:J, :],
                                             in0=hk[0][:, 0:J, :],
                                             in1=hk[2][:, 0:J, :])
                        qk = pe.tile([P, Jm3, 7], bf16, tag="qk")
                        nc.vector.tensor_mul(
                            out=qk[:, 0:J, :], in0=kvc[:, 0:J, 0:7],
                            in1=q_all[:, t:t + 1, :].to_broadcast([P, J, 7]))
                        # q.k dot via in-place halving over c (7 = 3+4)
                        nc.vector.tensor_add(out=qk[:, 0:J, 0:3],
                                             in0=qk[:, 0:J, 0:3],
                                             in1=qk[:, 0:J, 4:7])
                        nc.vector.tensor_add(out=qk[:, 0:J, 0:2],
                                             in0=qk[:, 0:J, 0:2],
                                             in1=qk[:, 0:J, 2:4])
                        al = pe.tile([P, Jm3, 1], f32, tag="al3")
                        nc.vector.tensor_add(out=al[:, 0:J, :],
                                             in0=qk[:, 0:J, 0:1],
                                             in1=qk[:, 0:J, 1:2])
                        exb = pe.tile([P, Jm3, 1], bf16, tag="exb3")
                        nc.scalar.activation(out=exb[:, 0:J, :],
                                             in_=al[:, 0:J, :], func=EXP,
                                             scale=scale3)
                        msg = pe.tile([P, Jm3, 8], bf16, tag="msg3")
                        nc.vector.scalar_tensor_tensor(
                            out=msg[:, 0:J, 7:8],
                            in0=cls[:, :, None].to_broadcast([P, J, 1]),
                            scalar=50.0, op0=STT.is_lt, op1=STT.mult,
                            in1=exb[:, 0:J, :])
                        nc.vector.tensor_mul(
                            out=msg[:, 0:J, 0:7], in0=kvc[:, 0:J, 7:14],
                            in1=msg[:, 0:J, 7:8].to_broadcast([P, J, 7]))
                        acc = pe.tile([P, (Jm3 + 1) // 2, 8], f32, tag="acc3")
                        tree_reduce(msg[:, 0:J, :], acc, J)
                        den = pe.tile([P, 1], f32, tag="den3")
                        nc.vector.tensor_add(out=den[:], in0=acc[:, 0, 7:8],
                                             in1=eps_sb[:, 0:1])
                        rden = pe.tile([P, 1], f32, tag="rden3")
                        nc.vector.reciprocal(out=rden[:], in_=den[:])
                        nc.vector.tensor_mul(out=x3st[:, bi, :],
                                             in0=acc[:, 0, 0:7],
                                             in1=rden[:].to_broadcast([P, 7]))
                        nc.vector.tensor_add(out=x3st[:, bi, :],
                                             in0=x3st[:, bi, :],
                                             in1=skip_sb[:, t, :])
                    nc.sync.dma_start(
                        out=out_loc.ap()[tb * P:(tb + BL) * P, :].rearrange(
                            "(b p) c -> p b c", p=P),
                        in_=x3st[:])
    nc.compile()
    return nc


# ---------------------------------------------------------------- kernel

def kernel(x, edge_index, W1, att_src1, att_dst1, b1, W2l, b2l, W2r, b2r,
           att2, b2, Wq, bq, Wk, bk, Wv, bv, Wskip, bskip):
    x = np.asarray(x, dtype=np.float32)
    edge_index = np.asarray(edge_index)
    W1 = np.asarray(W1, dtype=np.float64)
    att_src1 = np.asarray(att_src1, dtype=np.float64)
    att_dst1 = np.asarray(att_dst1, dtype=np.float64)

    src = np.asarray(edge_index[0], dtype=np.int64)
    dst = np.asarray(edge_index[1], dtype=np.int64)

    # degree-sorted node relabeling: rank r -> g = c*NPC + t*128 + p
    deg12 = np.bincount(dst, minlength=N) + 1
    order = np.argsort(-deg12, kind="stable")
    r = np.arange(NPAD)
    g_of_rank = ((r % TROW) // P) * NPC + (r // TROW) * P + (r % P)
    g_of_node = np.empty(N, dtype=np.int64)
    g_of_node[order] = g_of_rank[:N]

    gsrc = g_of_node[src]
    gdst = g_of_node[dst]
    gloop = g_of_node[np.arange(N)]
    gsrc12 = np.concatenate([gsrc, gloop])
    gdst12 = np.concatenate([gdst, gloop])

    deg12_g = np.zeros(NPAD, dtype=np.int64)
    deg12_g[g_of_node] = deg12
    deg3_g = np.zeros(NPAD, dtype=np.int64)
    deg3_g[g_of_node] = deg12 - 1

    geo12 = edge_geometry(gsrc12, gdst12, deg12_g)
    geo3 = edge_geometry(gsrc, gdst, deg3_g)

    # host-side E1 projections (as in the reference, fp64 for stability)
    W1r = W1.reshape(3, 4, 16)
    Asrc3 = (W1r * att_src1[None]).sum(-1)    # [3, 4]
    Adst3 = (W1r * att_dst1[None]).sum(-1)
    xg = np.zeros((NPAD, 3), dtype=np.float32)
    xg[g_of_node] = x
    asrc_g = (xg.astype(np.float64) @ Asrc3).astype(np.float32)
    adst_g = (xg.astype(np.float64) @ Adst3).astype(np.float32)

    # block-diagonal W1 for x1 = A @ Wblk: rows (h,k<3) -> cols (h, c)
    Wblk = np.zeros((16, 64), dtype=np.float32)
    for h in range(4):
        Wblk[h * 4:h * 4 + 3, h * 16:(h + 1) * 16] = W1[:, h * 16:(h + 1) * 16]

    rep = lambda v, w: np.broadcast_to(np.asarray(v, np.float32).reshape(1, w),
                                       (P, w)).copy()
    shared = {
        "Wblk": Wblk,
        "W2l": np.asarray(W2l, np.float32),
        "W2r": np.asarray(W2r, np.float32),
        "b1r": rep(b1, 64), "b2lr": rep(b2l, 32), "b2rr": rep(b2r, 32),
        "b2r_": rep(b2, 32),
        "att2r": rep(np.asarray(att2, np.float32).reshape(32), 32),
        "Wqkvs": np.concatenate([Wq, Wk, Wv, Wskip], axis=1).astype(np.float32),
        "bqkvsr": rep(np.concatenate([np.asarray(bq), np.asarray(bk),
                                      np.asarray(bv), np.asarray(bskip)]), 28),
    }
    in_maps = []
    for c in range(NCORES):
        m = dict(shared)
        d12 = pack_core(geo12, c, xg=xg, aprg=(asrc_g, adst_g))
        d3 = pack_core(geo3, c)
        m["xaf"] = d12["xa"]
        m["src12f"] = d12["src16"]
        m["cls12f"] = d12["cls"]
        m["src3f"] = d3["src16"]
        m["cls3f"] = d3["cls"]
        in_maps.append(m)

    nc = build(geo12["J"], geo12["off"], geo3["J"], geo3["off"])
    trace = bool(globals().get("_TRACE", False))
    res = run_bass_kernel_spmd(nc, in_maps, core_ids=list(range(NCORES)),
                               trace=trace)
    if trace:
        globals()["_LAST_RES"] = res
    outg = np.concatenate([res.results[c]["out_loc"] for c in range(NCORES)],
                          axis=0)
    return np.ascontiguousarray(outg[g_of_node]).astype(np.float32)
